# revision 1
# baseline (speedup 1.0000x reference)
"""Trainium2 Bass kernel for nn_MultiHeadAttention_47330539602717 (V2).

Math (per batch b, head h; q/k projections are dead code in the reference):
    vpT    = Wv^T @ v_b^T + bv            (1024, 4096)   [d on partitions]
    A_h    = vp_h @ i_h^T                 (4096 s, 128 q) [s on partitions]
    expA   = exp(A - 28)                  softmax1 numerator (shift exact)
    S1[s]  = sum_q expA[s, q]             free-axis reduce
    attn1  = expA / S1                    per-partition scalar multiply
    e2     = exp(9 * attn1)
    H[q',q]= sum_m e2[128m+q', q]         (torch raw .view fold)
    den2[q]= sum_{q'} H[q', q]
    w[q']  = sum_q H[q', q] / den2[q]
    x_h    = w @ i_h                      (64,)
    out_b  = concat_h(x_h) @ Wo + bo      (1, 1024)

Sharding: data-parallel over batch; core c handles batch b=c (16 heads).
Layout: STRUCTURE-2 — s on partitions for the whole softmax chain so that
softmax1 runs along the free axis (ACT accum/DVE reduce, per-partition
scalars) and the .view fold is a plain partition-aligned tile add.

Engine plan per core:
    PE   : big GEMM (bf16, 512 mm), A-matmuls (2-head block-diag rhs,
           256 mm of N=256), epilogue mms
    ACT  : exp1 + exp2 (Exp table stays loaded; bias=-28 / scale=9 fused)
    DVE  : S1 reduce (split with Pool), 1/S1, normalize (tensor_scalar
           per-partition 4x), fold adds (bf16 2x)
    Pool : PSUM->SBUF vpT copies with fused +bv, part of S1 reduces
    DMA  : vT/Wv/Wo bf16 streams (~13 MB)
"""

import sys

import numpy as np

sys.path.insert(0, "/opt/trn_rl_repo")

from contextlib import ExitStack

import concourse.bacc as bacc
import concourse.tile as tile
from concourse import mybir
from concourse.bass_utils import run_bass_kernel_spmd

F32 = mybir.dt.float32
BF16 = mybir.dt.bfloat16
EXP = mybir.ActivationFunctionType.Exp
AX = mybir.AxisListType.X
ADD = mybir.AluOpType.add
MULT = mybir.AluOpType.mult

B, LQ, S, D, H = 8, 128, 4096, 1024, 16
DK = D // H          # 64
KD = 8               # k blocks of 128 over D (contraction)
NCH = 512            # s per chunk
NCHK = S // NCH      # 8 chunks
NSUB = NCH // 128    # 4 subtiles of 128 s per chunk
SHIFT = 28.0
SMOOTH = 9.0


def build_program():
    nc = bacc.Bacc("TRN2", target_bir_lowering=False, debug=False,
                   num_devices=8)

    vT_d = nc.dram_tensor("vT", [D, S], BF16, kind="ExternalInput").ap()
    Wv_d = nc.dram_tensor("Wv", [D, D], BF16, kind="ExternalInput").ap()
    Wo_d = nc.dram_tensor("Wo", [D, D], BF16, kind="ExternalInput").ap()
    bv_d = nc.dram_tensor("bv", [128, KD], F32, kind="ExternalInput").ap()
    iTbd_d = nc.dram_tensor("iTbd", [128, KD, 256], BF16,
                            kind="ExternalInput").ap()
    iN_d = nc.dram_tensor("iN", [128, H, DK], BF16, kind="ExternalInput").ap()
    bo_d = nc.dram_tensor("bo", [1, D], F32, kind="ExternalInput").ap()
    onesc_d = nc.dram_tensor("onesc", [128, 1], BF16, kind="ExternalInput").ap()
    onesr_d = nc.dram_tensor("onesr", [1, 128], BF16, kind="ExternalInput").ap()
    out_d = nc.dram_tensor("out", [1, D], F32, kind="ExternalOutput").ap()

    with tile.TileContext(nc) as tc, ExitStack() as ctx:
        singles = ctx.enter_context(tc.tile_pool(name="singles", bufs=1))
        vtp = ctx.enter_context(tc.tile_pool(name="vtp", bufs=2))
        vpp = ctx.enter_context(tc.tile_pool(name="vpp", bufs=2))
        eap = ctx.enter_context(tc.tile_pool(name="eap", bufs=2))
        a9p = ctx.enter_context(tc.tile_pool(name="a9p", bufs=2))
        e2p = ctx.enter_context(tc.tile_pool(name="e2p", bufs=2))
        smp = ctx.enter_context(tc.tile_pool(name="smp", bufs=3))
        vp_ps = ctx.enter_context(
            tc.tile_pool(name="vp_ps", bufs=2, space="PSUM"))
        a_ps = ctx.enter_context(
            tc.tile_pool(name="a_ps", bufs=3, space="PSUM"))

        # ---- constants / weights ----
        # chunk-0 vT goes first on the DMA queue, and Wv is split per
        # k-chunk, so GEMM(0) k0 starts ~7us earlier (startup only;
        # steady-state streams untouched)
        vT0_sb = vtp.tile([128, KD, NCH], BF16, tag="vT")
        nc.sync.dma_start(
            out=vT0_sb,
            in_=vT_d[:, 0:NCH].rearrange("(k p) s -> p k s", p=128))
        Wv_sb = singles.tile([128, KD, D], BF16)      # k-chunk k at [:, k, :]
        for k in range(KD):
            nc.sync.dma_start(out=Wv_sb[:, k, :],
                              in_=Wv_d[128 * k:128 * (k + 1), :])
        iTbd_sb = singles.tile([128, KD, 256], BF16)  # block-diag i^T pairs
        nc.sync.dma_start(out=iTbd_sb, in_=iTbd_d)
        iN_sb = singles.tile([128, H, DK], BF16)      # natural i per head
        nc.sync.dma_start(out=iN_sb, in_=iN_d)
        bv_sb = singles.tile([128, KD], F32)          # bv[128*db+p] at [p, db]
        nc.sync.dma_start(out=bv_sb, in_=bv_d)
        bo_sb = singles.tile([1, D], F32)
        nc.sync.dma_start(out=bo_sb, in_=bo_d)
        onesc = singles.tile([128, 1], BF16)
        nc.sync.dma_start(out=onesc, in_=onesc_d)
        onesr = singles.tile([1, 128], BF16)
        nc.sync.dma_start(out=onesr, in_=onesr_d)
        Pfold = singles.tile([128, KD, 256], BF16)    # head 2j+t at [:, j, 128t:]
        nshift = singles.tile([128, 1], F32)          # exp1 bias = -28
        nc.vector.memset(nshift, -SHIFT)
        zbias = singles.tile([128, 1], F32)           # exp2 bias = 0
        nc.vector.memset(zbias, 0.0)

        # ---- main loop: V2 structure (GEMM then softmax per chunk;
        # cross-chunk overlap via pool buffer rotation) ----
        for c in range(NCHK):
            if c == 0:
                vT_sb = vT0_sb
            else:
                vT_sb = vtp.tile([128, KD, NCH], BF16, tag="vT")
                nc.sync.dma_start(
                    out=vT_sb,
                    in_=vT_d[:, c * NCH:(c + 1) * NCH].rearrange(
                        "(k p) s -> p k s", p=128))

            vpT_sb = vpp.tile([128, KD, NCH], BF16, tag="vp")
            for db in range(KD):
                vp_p = vp_ps.tile([128, NCH], F32, tag="vp")
                for k in range(KD):
                    nc.tensor.matmul(
                        vp_p,
                        lhsT=Wv_sb[:, k, db * 128:(db + 1) * 128],
                        rhs=vT_sb[:, k, :],
                        start=(k == 0), stop=(k == KD - 1),
                    )
                # PSUM -> SBUF bf16 with fused +bv (Identity shares the
                # ACT table set with Exp)
                nc.scalar.activation(
                    vpT_sb[:, db, :], vp_p,
                    mybir.ActivationFunctionType.Identity,
                    bias=bv_sb[:, db:db + 1])

            for m in range(NSUB):
                expA = eap.tile([128, KD, 256], BF16, tag="ea")
                attn9 = a9p.tile([128, KD, 256], BF16, tag="a9")
                e2 = e2p.tile([128, KD, 256], BF16, tag="e2")
                S1 = smp.tile([128, H], F32, tag="s1")
                RS1 = smp.tile([128, H], F32, tag="rs1")
                for g in range(2):
                    A_p = a_ps.tile([128, 4, 256], F32, tag="A")
                    for jj in range(4):
                        j = 4 * g + jj
                        nc.tensor.matmul(
                            A_p[:, jj, :],
                            lhsT=vpT_sb[:, j, m * 128:(m + 1) * 128],
                            rhs=iTbd_sb[:, j, :],
                            start=True, stop=True,
                        )
                    nc.scalar.activation(expA[:, 4 * g:4 * g + 4, :], A_p,
                                         EXP, bias=nshift)
                    red = expA[:, 4 * g:4 * g + 4, :].rearrange(
                        "p j (t q) -> p j t q", q=128)
                    nc.vector.tensor_reduce(
                        out=S1[:, 8 * g:8 * g + 8], in_=red, axis=AX, op=ADD)
                nc.vector.reciprocal_approx_fast(RS1, S1)
                for j in range(KD):
                    for t in range(2):
                        hs = 2 * j + t
                        nc.vector.tensor_scalar(
                            out=attn9[:, j, 128 * t:128 * (t + 1)],
                            in0=expA[:, j, 128 * t:128 * (t + 1)],
                            scalar1=RS1[:, hs:hs + 1], scalar2=None, op0=MULT)
                for g in range(2):
                    sl = slice(4 * g, 4 * g + 4)
                    nc.scalar.activation(e2[:, sl, :], attn9[:, sl, :],
                                         EXP, bias=zbias, scale=SMOOTH)
                    if c == 0 and m == 0:
                        nc.vector.tensor_copy(Pfold[:, sl, :], e2[:, sl, :])
                    else:
                        nc.vector.tensor_add(Pfold[:, sl, :],
                                             Pfold[:, sl, :], e2[:, sl, :])

        # stage Wo in the freed vT slots for the epilogue
        Wo_t = []
        for n in range(2):
            Wo_h = vtp.tile([128, KD, NCH], BF16, tag="vT")
            Wo_t.append(Wo_h)
            nc.sync.dma_start(
                out=Wo_h,
                in_=Wo_d[:, n * NCH:(n + 1) * NCH].rearrange(
                    "(k p) c -> p k c", p=128))

        # ---- epilogue ----
        den = singles.tile([1, KD, 256], F32)
        for n in range(4):
            den_p = vp_ps.tile([1, 2, 256], F32, tag="vp")
            nc.tensor.matmul(den_p, lhsT=onesc,
                             rhs=Pfold[:, 2 * n:2 * n + 2, :],
                             start=True, stop=True)
            nc.vector.tensor_copy(den[:, 2 * n:2 * n + 2, :], den_p)
        rden = singles.tile([1, KD, 256], F32)
        nc.vector.reciprocal_approx_fast(rden, den)
        rdenb = singles.tile([1, KD, 256], BF16)
        nc.vector.tensor_copy(rdenb, rden)

        w_sb = singles.tile([128, H], F32)
        for g in range(4):           # head quads: j in {2g, 2g+1}, t in {0,1}
            R_p = vp_ps.tile([128, 2, 256], F32, tag="vp")
            for jj in range(2):
                j = 2 * g + jj
                for t in range(2):
                    nc.tensor.matmul(
                        R_p[:, jj, 128 * t:128 * (t + 1)],
                        lhsT=onesr,
                        rhs=rdenb[:, j, 128 * t:128 * (t + 1)],
                        start=True, stop=True, skip_group_check=True,
                    )
            Hs = smp.tile([128, 2, 256], BF16, tag="hs")
            nc.vector.tensor_mul(Hs, Pfold[:, 2 * g:2 * g + 2, :], R_p)
            nc.vector.tensor_reduce(
                out=w_sb[:, 4 * g:4 * g + 4],
                in_=Hs.rearrange("p j (t q) -> p j t q", q=128),
                axis=AX, op=ADD)
        w_bf = singles.tile([128, H], BF16)
        nc.vector.tensor_copy(w_bf, w_sb)

        x_p = vp_ps.tile([128, KD], F32, tag="vp")
        for j in range(KD):
            for t in range(2):
                h = 2 * j + t
                nc.tensor.matmul(
                    x_p[64 * t:64 * t + 64, j:j + 1],
                    lhsT=iN_sb[:, h, :],
                    rhs=w_bf[:, h:h + 1],
                    start=True, stop=True, skip_group_check=True,
                )
        x_bf = singles.tile([128, KD], BF16)
        nc.vector.tensor_copy(x_bf, x_p)

        out_sb = singles.tile([1, D], F32)
        for n in range(2):
            o_p = a_ps.tile([1, NCH], F32, tag="A")
            for j in range(KD):
                nc.tensor.matmul(
                    o_p,
                    lhsT=x_bf[:, j:j + 1],
                    rhs=Wo_t[n][:, j, :],
                    start=(j == 0), stop=(j == KD - 1),
                )
            nc.vector.tensor_add(out_sb[:, n * NCH:(n + 1) * NCH], o_p,
                                 bo_sb[:, n * NCH:(n + 1) * NCH])
        nc.sync.dma_start(out=out_d, in_=out_sb)

    nc.compile()
    return nc


def make_in_maps(v, i, Wv, bv, Wo, bo):
    """Shard + lay out inputs per core (core c = batch c)."""
    import ml_dtypes
    bf = ml_dtypes.bfloat16
    v = np.asarray(v, np.float32)
    i = np.asarray(i, np.float32)
    Wv_b = np.ascontiguousarray(np.asarray(Wv, np.float32)).astype(bf)
    Wo_b = np.ascontiguousarray(np.asarray(Wo, np.float32)).astype(bf)
    bv = np.asarray(bv, np.float32)
    bo = np.ascontiguousarray(np.asarray(bo, np.float32)).reshape(1, D)
    bv_sb = np.ascontiguousarray(bv.reshape(KD, 128).T)          # [p, db]
    onesc = np.ones((128, 1), np.float32).astype(bf)
    onesr = np.ones((1, 128), np.float32).astype(bf)
    in_maps = []
    for b in range(B):
        hv = i[b * H:(b + 1) * H]                      # (16, 128, 64)
        iTbd = np.zeros((128, KD, 256), np.float32)
        for j in range(KD):
            iTbd[0:64, j, 0:128] = hv[2 * j].T         # head 2j
            iTbd[64:128, j, 128:256] = hv[2 * j + 1].T  # head 2j+1
        iN = np.ascontiguousarray(np.transpose(hv, (1, 0, 2)))  # (128,16,64)
        in_maps.append({
            "vT": np.ascontiguousarray(v[b].T).astype(bf),
            "Wv": Wv_b,
            "Wo": Wo_b,
            "bv": bv_sb,
            "iTbd": iTbd.astype(bf),
            "iN": iN.astype(bf),
            "bo": bo,
            "onesc": onesc,
            "onesr": onesr,
        })
    return in_maps


_NC_CACHE = None


def kernel(q, k, v, i, Wq, bq, Wk, bk, Wv, bv, Wo, bo):
    global _NC_CACHE
    if _NC_CACHE is None:
        _NC_CACHE = build_program()
    nc = _NC_CACHE
    in_maps = make_in_maps(v, i, Wv, bv, Wo, bo)
    res = run_bass_kernel_spmd(nc, in_maps, list(range(8)))
    rows = [res.results[c]["out"].reshape(1, D) for c in range(B)]
    return np.stack(rows, axis=0).astype(np.float32)  # (8, 1, 1024)


if __name__ == "__main__":
    build_program()
    print("compiled OK")

